# revision 1
# baseline (speedup 1.0000x reference)
"""Trainium2 Bass kernel for ColorHistogramLoss.

Reference computation:
  brightness = mean(target, axis=1)           # [B,1,H,W]
  mask = brightness > 0.4
  soft 16-bin Gaussian histograms of pred/target per (b, c), masked,
  normalized; loss = mean |pred_hist - target_hist|.

Kernel strategy (8 NeuronCores, data-parallel over batch B=8):
  Each core processes one image pair (pred[b], target[b]) [3,512,512] each.

  Math identity per bin k (center c = k/15):
    exp(-128*(x-c)^2) = exp( (256*c)*x + (-128*x^2 - 128*c^2) )
  so with v = -128 x^2 (+ mask offset), one fused DVE/GPSIMD
  scalar_tensor_tensor builds e_k = x*(256c) + vm per bin and one ScalarE
  activation evaluates exp with per-bin bias -128c^2 and a fused
  per-partition reduction (accum_out).  Masked-out pixels get
  vm ~= -50000 -> exp == 0 exactly in f32.

  Layout: channels are PAIR-STACKED on the partition axis: a [128, 4096]
  tile holds channel A on partitions 0..63 and channel B on 64..127
  (each channel flat 262144 = 64 x 4096).  This halves the ScalarE
  instruction count (its per-instruction overhead is ~352 cycles).
  All steady-state per-bin work runs on DVE + ScalarE; see the engine
  notes on the config knobs below.

  Output per core: stats [128, 48] per-partition histogram partials;
  the tiny normalize / L1 / mean finish runs on host (only a scalar
  "all-reduce" is needed).

Measured (axon-tunneled trn2, repeat-slope method, 8 cores in parallel):
  ~240 us per kernel execution; relative error vs reference 7.6e-7.
  Engine balance (cost model): DVE ~94% busy (51 fused scalar_tensor_tensor
  ops), ScalarE ~77% (48 exp+accum instructions = the 16-exps-per-element
  floor), GPSIMD/PE idle.  Offloading work to GPSIMD measurably degraded
  DVE throughput on HW in every configuration tried, so everything
  steady-state runs on DVE + ScalarE.
"""

from contextlib import ExitStack

import numpy as np

import concourse.bass as bass
import concourse.tile as tile
from concourse import bacc, mybir
from concourse.bass_utils import run_bass_kernel_spmd

N_CORES = 8
C = 3
H = 512
W = 512
HW = H * W          # 262144
P = 128
HP = 64             # partitions per channel in a stacked pair
FP = HW // HP       # 4096
NB = 16
NPAIR = 3           # (pred0,pred1), (pred2,target0), (target1,target2)
BIGNEG = -50000.0
F32 = mybir.dt.float32

# bins computed on GPSIMD as "echo" adds e_k = e_{k-1} + (256/15)*x off a
# DVE-produced predecessor (GPSIMD supports TensorTensor but not
# scalar_tensor_tensor); no two adjacent, none may be 1 less than another.
# Empirically GPSIMD concurrency degrades DVE on HW (shared SBUF ports),
# so this is best left empty.
GPSIMD_BINS = ()
# bins computed entirely on ScalarE as Square(x'-c) then Exp (no DVE work);
# x' is the mask-offset input built once per pair.  Empirically slower on
# HW than keeping ScalarE on pure Exp duty (extra ACT ops + table churn).
ACT_SQ_BINS = ()
PREP_POS = 12
# compute the brightness mask on GPSIMD (idle during the fill phase) reading
# the stacked tiles directly; requires Pool to accept cross-partition-base
# inputs, which walrus may reject
MASK_ON_GPS = False
# compute vm = v + off2 on GPSIMD.  Every measured GPSIMD offload (echo
# bins, mask, vm) slowed the kernel on HW — concurrent Pool traffic
# degrades DVE throughput — so steady-state work stays on DVE/ScalarE.
VM_ON_GPS = False
# tile-pool buffer depths (SBUF budget: see _kernel_body)
E_BUFS = 4
VM_BUFS = 2
# how many of the next pair's leading e_k tiles are pre-emitted on DVE during
# the current pair's tail.  0: DVE is the bottleneck, so ScalarE's small
# pair-transition gaps are slack, and pre-emission only disturbs slot reuse.
PRE_BINS = 0


def _kernel_body(
    ctx: ExitStack, tc: "tile.TileContext", stats_d, pred_d, target_d, repeat=1
):
    nc = tc.nc
    stacks = ctx.enter_context(tc.tile_pool(name="stacks", bufs=1))
    maskp = ctx.enter_context(tc.tile_pool(name="maskp", bufs=1))
    vpool = ctx.enter_context(tc.tile_pool(name="vpool", bufs=1))
    epool = ctx.enter_context(tc.tile_pool(name="epool", bufs=E_BUFS))
    wpool = ctx.enter_context(tc.tile_pool(name="wpool", bufs=1, space="PSUM"))
    spool = ctx.enter_context(tc.tile_pool(name="spool", bufs=1))
    pools = (stacks, maskp, vpool, epool, wpool, spool)

    # Per-bin ACT bias constants (ACT bias must be an AP): bias_k = -128*c_k^2
    # for the exp path, cbias_k = -c_k for the Square path.  Constant across
    # passes, so built once.
    bias_t = spool.tile([P, NB], F32, tag="bias")
    cbias_t = spool.tile([P, NB], F32, tag="cbias")
    for k in range(NB):
        ck = k / 15.0
        nc.gpsimd.memset(bias_t[:, k : k + 1], -128.0 * ck * ck)
        nc.gpsimd.memset(cbias_t[:, k : k + 1], -ck)

    for _ in range(repeat):
        _emit_pass(tc, pools, bias_t, cbias_t, stats_d, pred_d, target_d)


def _emit_pass(
    tc: "tile.TileContext", pools, bias_t, cbias_t, stats_d, pred_d, target_d
):
    nc = tc.nc
    add = mybir.AluOpType.add
    mult = mybir.AluOpType.mult
    is_le = mybir.AluOpType.is_le
    stacks, maskp, vpool, epool, wpool, spool = pools

    def chan_ap(dram, c):
        # [64, 4096] flat view of one channel
        return dram[c].rearrange("(q g) -> q g", q=HP)

    # Stacked pair tiles: [128, 4096], channel A on partitions 0..63, B on
    # 64..127.  The target channels (mask inputs) are loaded first.
    pair_srcs = [
        (chan_ap(target_d, 1), chan_ap(target_d, 2)),
        (chan_ap(pred_d, 2), chan_ap(target_d, 0)),
        (chan_ap(pred_d, 0), chan_ap(pred_d, 1)),
    ]
    # The mask path gates everything, so its DMAs go first: pair0 plus
    # base-0 re-reads of t0/t2 (DVE needs both inputs at the same base
    # partition; the scratch tiles are borrowed from the e pool).
    pair_tiles = []
    t = stacks.tile([P, FP], F32, tag="pair0")
    nc.sync.dma_start(out=t[:HP, :], in_=pair_srcs[0][0])
    nc.sync.dma_start(out=t[HP:, :], in_=pair_srcs[0][1])
    pair_tiles.append(t)
    t1 = pair_tiles[0][:HP, :]
    m2 = epool.tile([P, FP], F32, tag="e")
    nc.sync.dma_start(out=m2[:HP, :], in_=chan_ap(target_d, 2))
    m0 = epool.tile([P, FP], F32, tag="e")
    nc.sync.dma_start(out=m0[:HP, :], in_=chan_ap(target_d, 0))
    for i, (a_ap, b_ap) in enumerate(pair_srcs[1:], start=1):
        t = stacks.tile([P, FP], F32, tag=f"pair{i}")
        nc.sync.dma_start(out=t[:HP, :], in_=a_ap)
        nc.sync.dma_start(out=t[HP:, :], in_=b_ap)
        pair_tiles.append(t)
    off2 = maskp.tile([P, FP], F32, tag="off2")
    s = off2[:HP, :]  # lower half doubles as scratch for the brightness sum
    meng = nc.gpsimd if MASK_ON_GPS else nc.vector
    meng.tensor_tensor(out=s, in0=m0[:HP, :], in1=t1, op=add)
    meng.tensor_tensor(out=s, in0=s, in1=m2[:HP, :], op=add)
    meng.tensor_scalar(
        out=s, in0=s, scalar1=1.2, scalar2=BIGNEG, op0=is_le, op1=mult
    )
    # replicate to upper half (cross-partition read is allowed)
    meng.tensor_scalar(
        out=off2[HP:, :], in0=s, scalar1=1.0, scalar2=None, op0=mult
    )

    stats_t = spool.tile([P, NPAIR * NB], F32)

    for k in GPSIMD_BINS:
        assert k - 1 not in GPSIMD_BINS and k >= 1

    def emit_prep(x):
        """v, vm (+xc, xm as configured) for one pair."""
        v = epool.tile([P, FP], F32, tag="e")
        nc.vector.scalar_tensor_tensor(
            out=v[:], in0=x[:], scalar=-128.0, in1=x[:], op0=mult, op1=mult
        )
        xc = None
        if GPSIMD_BINS:
            # xc = (256/15) * x, the exponent increment for GPSIMD echo bins
            xc = vpool.tile([P, FP], F32, tag="xc", bufs=2)
            nc.vector.tensor_scalar(
                out=xc[:], in0=x[:], scalar1=256.0 / 15.0, scalar2=None, op0=mult
            )
        vm = vpool.tile([P, FP], F32, tag="vm", bufs=VM_BUFS)
        veng = nc.gpsimd if VM_ON_GPS else nc.vector
        veng.tensor_tensor(out=vm[:], in0=v[:], in1=off2[:], op=add)
        xm = None
        if ACT_SQ_BINS:
            # xm = x + 100 where masked out (exp(-128*(xm-c)^2) == 0 there)
            xm = vpool.tile([P, FP], F32, tag="xm", bufs=2)
            nc.vector.scalar_tensor_tensor(
                out=xm[:], in0=off2[:], scalar=-0.002, in1=x[:], op0=mult, op1=add
            )
        return vm, xc, xm

    preps = [emit_prep(pair_tiles[0])]  # pair0 prep up front

    dve_bins = [
        k for k in range(1, NB) if k not in GPSIMD_BINS and k not in ACT_SQ_BINS
    ]
    bin_seq = [0] + dve_bins
    for k in range(1, NB):
        if k in GPSIMD_BINS:
            bin_seq.insert(k, k)  # echo bins need ascending placement
    bin_seq = bin_seq + [k for k in sorted(ACT_SQ_BINS)]

    def emit_stt(x, vm, k):
        e = epool.tile([P, FP], F32, tag="e")
        nc.vector.scalar_tensor_tensor(
            out=e[:], in0=x[:], scalar=256.0 * (k / 15.0), in1=vm[:],
            op0=mult, op1=add,
        )
        return e

    pre_emitted = [dict() for _ in pair_tiles]  # pair -> {k: e tile}

    for pi, x in enumerate(pair_tiles):
        vm, xc, xm = preps[pi]
        prev = vm
        for pos, k in enumerate(bin_seq):
            if k == 0:
                e = vm
            elif k in pre_emitted[pi]:
                e = pre_emitted[pi][k]
            elif k in GPSIMD_BINS:
                e = epool.tile([P, FP], F32, tag="e")
                nc.gpsimd.tensor_tensor(out=e[:], in0=prev[:], in1=xc[:], op=add)
            elif k in ACT_SQ_BINS:
                sqt = epool.tile([P, FP], F32, tag="e")
                nc.scalar.activation(
                    out=sqt[:],
                    in_=xm[:],
                    func=mybir.ActivationFunctionType.Square,
                    bias=cbias_t[:, k : k + 1],
                    scale=1.0,
                )
                e = sqt
            else:
                e = emit_stt(x, vm, k)
            prev = e
            if pos == PREP_POS and pi + 1 < len(pair_tiles):
                # software-pipeline: emit next pair's prep mid-stream so its
                # vm is ready the moment this pair's bins finish
                preps.append(emit_prep(pair_tiles[pi + 1]))
            if (
                PREP_POS < pos <= PREP_POS + PRE_BINS
                and pi + 1 < len(pair_tiles)
            ):
                # pre-emit the next pair's leading stt bins so ScalarE never
                # starves across the pair boundary
                nk = pos - PREP_POS
                if nk < NB and nk in bin_seq and nk not in GPSIMD_BINS \
                        and nk not in ACT_SQ_BINS:
                    pre_emitted[pi + 1][nk] = emit_stt(
                        pair_tiles[pi + 1], preps[pi + 1][0], nk
                    )
            w = wpool.tile([P, FP], F32, tag="w")
            nc.scalar.activation(
                out=w[:],
                in_=e[:],
                func=mybir.ActivationFunctionType.Exp,
                bias=0.0 if k in ACT_SQ_BINS else bias_t[:, k : k + 1],
                scale=-128.0 if k in ACT_SQ_BINS else 1.0,
                accum_out=stats_t[:, pi * NB + k : pi * NB + k + 1],
            )

    nc.sync.dma_start(out=stats_d[:], in_=stats_t[:])


def build_nc(repeat=1):
    nc = bacc.Bacc(
        "TRN2", target_bir_lowering=False, debug=False, num_devices=N_CORES
    )
    pred = nc.dram_tensor("pred", [C, HW], F32, kind="ExternalInput").ap()
    target = nc.dram_tensor("target", [C, HW], F32, kind="ExternalInput").ap()
    stats = nc.dram_tensor("stats", [P, NPAIR * NB], F32, kind="ExternalOutput").ap()
    with tile.TileContext(nc) as tc:
        with ExitStack() as ctx:
            _kernel_body(ctx, tc, stats, pred, target, repeat=repeat)
    nc.compile()
    return nc


_NC_CACHE = {}


def _get_nc():
    if "nc" not in _NC_CACHE:
        _NC_CACHE["nc"] = build_nc()
    return _NC_CACHE["nc"]


def stats_to_hists(stats):
    """[128, 48] per-core partials -> hist [2, C, NB] (pred, target) f64."""
    lo = stats[:HP].astype(np.float64).sum(axis=0).reshape(NPAIR, NB)
    hi = stats[HP:].astype(np.float64).sum(axis=0).reshape(NPAIR, NB)
    hist = np.empty((2, C, NB), np.float64)
    hist[1, 1] = lo[0]  # target c1
    hist[1, 2] = hi[0]  # target c2
    hist[0, 2] = lo[1]  # pred c2
    hist[1, 0] = hi[1]  # target c0
    hist[0, 0] = lo[2]  # pred c0
    hist[0, 1] = hi[2]  # pred c1
    return hist


def finish_on_host(stats_list):
    """stats_list: per-core [128, 48] f32 partials -> scalar f32 loss."""
    diffs = []
    for stats in stats_list:
        hist = stats_to_hists(stats)
        hist_n = hist / (hist.sum(axis=-1, keepdims=True) + 1e-7)
        diffs.append(np.abs(hist_n[0] - hist_n[1]))
    return np.array(np.mean(np.stack(diffs)), dtype=np.float32)


def run(pred, target, **spmd_kwargs):
    nc = _get_nc()
    pred = np.ascontiguousarray(np.asarray(pred, dtype=np.float32))
    target = np.ascontiguousarray(np.asarray(target, dtype=np.float32))
    assert pred.shape == (N_CORES, C, H, W), pred.shape
    in_maps = [
        {
            "pred": pred[b].reshape(C, HW),
            "target": target[b].reshape(C, HW),
        }
        for b in range(N_CORES)
    ]
    res = run_bass_kernel_spmd(nc, in_maps, core_ids=list(range(N_CORES)), **spmd_kwargs)
    loss = finish_on_host([res.results[b]["stats"] for b in range(N_CORES)])
    return loss, res


def kernel(pred, target):
    loss, _ = run(pred, target)
    return loss



# revision 8
# speedup vs baseline: 2.1035x; 2.1035x over previous
"""Trainium2 Bass kernel for ColorHistogramLoss.

Reference computation:
  brightness = mean(target, axis=1)           # [B,1,H,W]
  mask = brightness > 0.4
  soft 16-bin Gaussian histograms of pred/target per (b, c), masked,
  normalized; loss = mean |pred_hist - target_hist|.

Kernel strategy (8 NeuronCores, data-parallel over batch B=8), v2:
  Each core processes one image pair (pred[b], target[b]) [3,512,512].

  Instead of evaluating 16 exps per element (ScalarE-bound at ~178us)
  or 16 DVE scalar_tensor_tensor ops per element (DVE-bound at ~225us,
  the v1 baseline), exploit the multiplicative structure of the
  Gaussian row: with w_k = exp(-128(x - k/15)^2),

      w_{k+1} = w_k * r_up * qup_k,   r_up = exp((256/15)x - b)
      w_{k-1} = w_k * r_dn * qdn_k,   r_dn = exp(-(256/15)x - b)

  (qup/qdn are per-step compile-time scalars).  So the whole 16-bin row
  costs 2 ScalarE exps (r_up/r_dn) + 1 fp16 DVE stt per derived bin.
  fp16 (not bf16) is needed for precision (chain error compounds), but
  fp16's narrow range underflows mid-chain, so the row is split into
  two 8-bin chains, each freshly started at its center bin (4 and 12)
  with w_start = exp(-128*Square(xm - c) + 13*ln2) (ScalarE Square+Exp)
  and run bidirectionally.  The 2^13 start scale + e^{-+6} r-tile
  rescales keep every stored fp16 value in [6e-8, 65504]; host divides
  the scale back out.  Verified numerically: loss rel err ~1e-4.

  The brightness mask folds in as xm = x + 100*(1-m): masked elements
  sit ~100 away from every bin center, so every chain start underflows
  to exactly 0 and the recurrence keeps them at 0 in all bins.

  Per-bin accumulation runs on the otherwise-idle TensorEngine: a
  [128,2] ones stationary (one column per stacked channel half) sums
  each w_k tile into PSUM rows (32*pair + 2*bin + half) via 8
  accumulating 512-column matmuls.  One final DVE tensor_reduce
  collapses PSUM [96,512] -> [96,1] which is DMA'd out; the tiny
  normalize / L1 / mean finish runs on host.

  Layout: channels PAIR-STACKED on the partition axis as in v1: a
  [128, 4096] tile holds channel A on partitions 0..63, B on 64..127.

  Predicted engine busy per core: DVE ~118us (bound), PE ~83us,
  ScalarE ~67us, vs v1's DVE ~225us.
"""

from contextlib import ExitStack
import math

import numpy as np

import concourse.bass as bass
import concourse.tile as tile
from concourse import bacc, mybir
from concourse.bass_utils import run_bass_kernel_spmd

N_CORES = 8
C = 3
H = 512
W = 512
HW = H * W          # 262144
P = 128
HP = 64             # partitions per channel in a stacked pair
FP = HW // HP       # 4096
NB = 16
NPAIR = 3           # (t1,t2), (p2,t0), (p0,p1)
F32 = mybir.dt.float32
F16 = mybir.dt.float16

BETA = 128.0 / 225.0          # exp(-128(x-k/15)^2) = exp(-BETA (15x-k)^2)
A = 256.0 / 15.0              # d/dx of the up-ratio exponent
RS = 6.0                      # r-tile rescale (fp16 range)
SC = 13.0 * math.log(2.0)     # chain-start scale 2^13
CHAINS = ((4, 0, 7), (12, 8, 15))   # (start bin, lo, hi)
MM_CHUNK = 512                # matmul moving free-dim (PSUM bank = 512 f32)
STATS_ROWS = 2 * NPAIR * NB   # 96


def _kernel_body(ctx, tc, stats_d, pred_d, target_d, repeat=1):
    nc = tc.nc
    stacks = ctx.enter_context(tc.tile_pool(name="stacks", bufs=1))
    maskp = ctx.enter_context(tc.tile_pool(name="maskp", bufs=1))
    scr = ctx.enter_context(tc.tile_pool(name="scr", bufs=1))
    vpool = ctx.enter_context(tc.tile_pool(name="vpool", bufs=2))
    epool = ctx.enter_context(tc.tile_pool(name="epool", bufs=6))
    ppool = ctx.enter_context(tc.tile_pool(name="ppool", bufs=1, space="PSUM"))
    spool = ctx.enter_context(tc.tile_pool(name="spool", bufs=1))
    pools = (stacks, maskp, scr, vpool, epool, ppool, spool)

    # Per-bin [128, 32] fp16 stationaries: bin k has ones at
    # (partitions 0..63, col 2k) and (64..127, col 2k+1), zeros elsewhere.
    # Matmul out [32, 512] lands at PSUM base 32*pair (base must be
    # 0/32/64); the zero columns contribute 0 to the other bins' rows.
    ones_k = []
    for k in range(NB):
        o = spool.tile([P, 2 * NB], F16, tag=f"ones{k}")
        nc.gpsimd.memset(o[:], 0.0)
        nc.gpsimd.memset(o[:HP, 2 * k : 2 * k + 1], 1.0)
        nc.gpsimd.memset(o[HP:, 2 * k + 1 : 2 * k + 2], 1.0)
        ones_k.append(o)

    # ACT bias constants must be APs: one [128, 1] column per value.
    bias_vals = [-BETA - RS, -BETA + RS, SC] + [-s0 / 15.0 for s0, _, _ in CHAINS]
    biases = spool.tile([P, len(bias_vals)], F32, tag="biases")
    bias_ap = {}
    for i, v in enumerate(bias_vals):
        nc.gpsimd.memset(biases[:, i : i + 1], v)
        bias_ap[v] = biases[:, i : i + 1]

    for _ in range(repeat):
        _emit_pass(ctx, tc, pools, ones_k, bias_ap, stats_d, pred_d, target_d)


def _emit_pass(ctx, tc, pools, ones_k, bias_ap, stats_d, pred_d, target_d):
    nc = tc.nc
    add = mybir.AluOpType.add
    mult = mybir.AluOpType.mult
    is_le = mybir.AluOpType.is_le
    stacks, maskp, scr, vpool, epool, ppool, spool = pools

    def chan_ap(dram, c):
        return dram[c].rearrange("(q g) -> q g", q=HP)

    pair_srcs = [
        (chan_ap(target_d, 1), chan_ap(target_d, 2)),
        (chan_ap(pred_d, 2), chan_ap(target_d, 0)),
        (chan_ap(pred_d, 0), chan_ap(pred_d, 1)),
    ]
    # pair0 (mask inputs) first, plus base-0 re-reads of t0/t2 so the
    # brightness sum has all three channels at the same base partition.
    pair_tiles = []
    t = stacks.tile([P, FP], F32, tag="pair0")
    nc.sync.dma_start(out=t[:HP, :], in_=pair_srcs[0][0])
    nc.sync.dma_start(out=t[HP:, :], in_=pair_srcs[0][1])
    pair_tiles.append(t)
    m0 = scr.tile([P, FP], F32, tag="m0")
    nc.sync.dma_start(out=m0[:HP, :], in_=chan_ap(target_d, 0))
    m2 = scr.tile([P, FP], F32, tag="m2")
    nc.sync.dma_start(out=m2[:HP, :], in_=chan_ap(target_d, 2))
    for i, (a_ap, b_ap) in enumerate(pair_srcs[1:], start=1):
        t = stacks.tile([P, FP], F32, tag=f"pair{i}")
        nc.sync.dma_start(out=t[:HP, :], in_=a_ap)
        nc.sync.dma_start(out=t[HP:, :], in_=b_ap)
        pair_tiles.append(t)

    # off2 = 100 where masked out (brightness sum <= 1.2), else 0.
    off2 = maskp.tile([P, FP], F32, tag="off2")
    s = off2[:HP, :]
    nc.vector.tensor_tensor(out=s, in0=m0[:HP, :], in1=pair_tiles[0][:HP, :], op=add)
    nc.vector.tensor_tensor(out=s, in0=s, in1=m2[:HP, :], op=add)
    nc.vector.tensor_scalar(
        out=s, in0=s, scalar1=1.2, scalar2=100.0, op0=is_le, op1=mult
    )
    nc.sync.dma_start(out=off2[HP:, :], in_=s)  # replicate to upper half

    psum = ppool.tile([STATS_ROWS, MM_CHUNK], F32, tag="psum")
    nchunk = FP // MM_CHUNK
    # per pair-region: 16 bins x nchunk accumulating matmuls; first resets,
    # last closes the group.
    mm_total = NB * nchunk
    mm_count = [0, 0, 0]

    def mm(w, pi, k):
        base = 32 * pi
        for c in range(nchunk):
            n = mm_count[pi]
            nc.tensor.matmul(
                out=psum[base : base + 2 * NB, :],
                lhsT=ones_k[k][:],
                rhs=w[:, c * MM_CHUNK : (c + 1) * MM_CHUNK],
                start=(n == 0),
                stop=(n == mm_total - 1),
            )
            mm_count[pi] = n + 1

    for pi, x in enumerate(pair_tiles):
        xm = vpool.tile([P, FP], F16, tag="xm")
        nc.vector.tensor_tensor(out=xm[:], in0=x[:], in1=off2[:], op=add)
        r_up = vpool.tile([P, FP], F16, tag="r_up")
        nc.scalar.activation(
            out=r_up[:], in_=x[:], func=mybir.ActivationFunctionType.Exp,
            bias=bias_ap[-BETA - RS], scale=A,
        )
        r_dn = vpool.tile([P, FP], F16, tag="r_dn")
        nc.scalar.activation(
            out=r_dn[:], in_=x[:], func=mybir.ActivationFunctionType.Exp,
            bias=bias_ap[-BETA + RS], scale=-A,
        )
        for s0, lo, hi in CHAINS:
            sq = epool.tile([P, FP], F16, tag="e")
            nc.scalar.activation(
                out=sq[:], in_=xm[:], func=mybir.ActivationFunctionType.Square,
                bias=bias_ap[-s0 / 15.0], scale=1.0,
            )
            w0 = epool.tile([P, FP], F16, tag="e")
            nc.scalar.activation(
                out=w0[:], in_=sq[:], func=mybir.ActivationFunctionType.Exp,
                bias=bias_ap[SC], scale=-128.0,
            )
            mm(w0, pi, s0)
            wp = w0
            for k in range(s0, hi):      # ascend
                wn = epool.tile([P, FP], F16, tag="e")
                nc.vector.scalar_tensor_tensor(
                    out=wn[:], in0=r_up[:], scalar=math.exp(-2.0 * BETA * k + RS),
                    in1=wp[:], op0=mult, op1=mult,
                )
                mm(wn, pi, k + 1)
                wp = wn
            wp = w0
            for k in range(s0, lo, -1):  # descend
                wn = epool.tile([P, FP], F16, tag="e")
                nc.vector.scalar_tensor_tensor(
                    out=wn[:], in0=r_dn[:], scalar=math.exp(2.0 * BETA * k - RS),
                    in1=wp[:], op0=mult, op1=mult,
                )
                mm(wn, pi, k - 1)
                wp = wn

    stats_t = spool.tile([STATS_ROWS, 1], F32, tag="stats")
    nc.vector.tensor_reduce(
        out=stats_t[:], in_=psum[:], axis=mybir.AxisListType.X,
        op=mybir.AluOpType.add,
    )
    nc.sync.dma_start(out=stats_d[:], in_=stats_t[:])


def build_nc(repeat=1):
    nc = bacc.Bacc(
        "TRN2", target_bir_lowering=False, debug=False, num_devices=N_CORES
    )
    pred = nc.dram_tensor("pred", [C, HW], F32, kind="ExternalInput").ap()
    target = nc.dram_tensor("target", [C, HW], F32, kind="ExternalInput").ap()
    stats = nc.dram_tensor(
        "stats", [STATS_ROWS, 1], F32, kind="ExternalOutput"
    ).ap()
    with tile.TileContext(nc) as tc:
        with ExitStack() as ctx:
            _kernel_body(ctx, tc, stats, pred, target, repeat=repeat)
    nc.compile()
    return nc


_NC_CACHE = {}


def _get_nc():
    if "nc" not in _NC_CACHE:
        _NC_CACHE["nc"] = build_nc()
    return _NC_CACHE["nc"]


# stats row -> (which hist 0=pred/1=target, channel): row = 32*pair + 2*bin + half
_PAIR_CHANNELS = [((1, 1), (1, 2)), ((0, 2), (1, 0)), ((0, 0), (0, 1))]


def stats_to_hists(stats):
    """[96, 1] per-core sums -> hist [2, C, NB] (pred, target) f64."""
    v = stats.reshape(NPAIR, NB, 2).astype(np.float64)
    hist = np.empty((2, C, NB), np.float64)
    for p in range(NPAIR):
        for half in range(2):
            which, ch = _PAIR_CHANNELS[p][half]
            hist[which, ch] = v[p, :, half]
    return hist


def finish_on_host(stats_list):
    diffs = []
    for stats in stats_list:
        hist = stats_to_hists(stats)
        hist_n = hist / (hist.sum(axis=-1, keepdims=True) + 1e-7)
        diffs.append(np.abs(hist_n[0] - hist_n[1]))
    return np.array(np.mean(np.stack(diffs)), dtype=np.float32)


def run(pred, target, **spmd_kwargs):
    nc = _get_nc()
    pred = np.ascontiguousarray(np.asarray(pred, dtype=np.float32))
    target = np.ascontiguousarray(np.asarray(target, dtype=np.float32))
    assert pred.shape == (N_CORES, C, H, W), pred.shape
    in_maps = [
        {
            "pred": pred[b].reshape(C, HW),
            "target": target[b].reshape(C, HW),
        }
        for b in range(N_CORES)
    ]
    res = run_bass_kernel_spmd(
        nc, in_maps, core_ids=list(range(N_CORES)), **spmd_kwargs
    )
    loss = finish_on_host([res.results[b]["stats"] for b in range(N_CORES)])
    return loss, res


def kernel(pred, target):
    loss, _ = run(pred, target)
    return loss
